# revision 41
# baseline (speedup 1.0000x reference)
"""FANMoE HyperNet layer on 8 TRN2 NeuronCores.

Strategy: the reference materializes delta = h @ hW2 (512 x 394240, ~800MB).
Algebraically the einsum with x collapses per hypernet unit k:
    dyn[b, o] = sum_k h[b,k] * (x @ W2_k)[b, o]
where W2_k is the (IN, N*(DP+DN)) slice of hW2 for unit k. We shard the 8
cores as 4 expert-pairs x 2 batch-halves: each core handles 2 experts
(o-width 384) for 256 samples, streaming its hW2 slice (1/4 of columns)
once from HBM as fp16 (single hi term; measured ~5e-4 rel fro error,
gate is 2e-2; x and base weights also fp16).

Main loop: supergroups of SG=8 units; the PE runs 16 matmuls per
(group, batch-tile) phase with 8-deep stationary reuse (LDWEIGHTS
amortization / HAM warmth dominate PE throughput: 134ns/MM at reuse-8 vs
281ns at reuse-4). The per-sample combination over k (the drain of each
(128,384) PSUM plane, scaled by h[:,k] per-partition) is the structural
bottleneck (~53us/core); it is split across both vector-capable engines:
  - NSTT[bt] units per phase drain via DVE scalar_tensor_tensor straight
    into the fp32 accumulator (~840ns/op),
  - the rest drain via ACT scaled copies (activation Copy with
    scale=h[:,k]) PSUM->SBUF fp16 (~660ns/op), then ONE grouped DVE
    tensor_add per phase (16-bit 2x mode, ~250ns/plane) accumulates them;
    slot-planes are tree-merged after the loop.
Epilogue: sin/cos via range reduction r = v - 2pi*rint(v/2pi [+1/4]),
cos folds pi/2 into the ACT Sin bias; per-expert gating collapses to 2
DVE ops via a [cv|sv|nn] staging layout matching the output columns.

Host-side work is limited to layout prep (transposes, dtype splits,
slicing), sharding, and summing the per-core partial outputs.
"""
import math

import numpy as np

import concourse.bass as bass
import concourse.tile as tile
from concourse import mybir, bacc
from concourse.masks import make_identity

B, IN, OUT, COND, N, H = 512, 256, 256, 128, 8, 64
DP = 64
DN = 128
TPE = IN * DP + IN * DN + DN
BH = B // 2          # samples per core (batch half)
NBT = BH // 128      # 128-row tiles per core
W = 2 * (DP + DN)    # per-core output width: 2 experts x 192 = 384
SG = 8               # hypernet units per DMA supergroup / PSUM phase
NSG = H // SG        # 8 supergroups
NSTT = (2, 3)        # per-phase count of direct-DVE-stt units, by bt
MMORD = (5, 6, 7, 0, 1, 2, 3, 4)   # MM issue order within a phase
GUARDS = False       # extra range-reduction guard ops in sin/cos
dt = mybir.dt
F32 = dt.float32
F16 = dt.float16
AF = mybir.ActivationFunctionType
OP = mybir.AluOpType
INV2PI = 1.0 / (2.0 * math.pi)
N2PI = -2.0 * math.pi

_cache = {}


def _build(repeat_main=1, ablate=()):
    nc = bacc.Bacc("TRN2", target_bir_lowering=False, debug=False)

    def din(name, shape, dty=F32):
        return nc.dram_tensor(name, shape, dty, kind="ExternalInput").ap()

    xh16 = din("xh16", (2, 128, BH), F16)
    condT = din("condT", (COND, BH))
    # per supergroup g of SG units: (128, SG*768) fp16, per k: [m0 | m1]
    # with m_ic = [wp_ic (128 cols) | wn_ic (256 cols)]
    w2g = din("w2g", (NSG, 128, SG * 768), F16)
    w2b = din("w2b", (2, H + 1, DN))
    wbase = din("wbase", (2, 128, W), F16)
    hW1 = din("hW1", (COND, H))
    hb1 = din("hb1", (1, H))
    gW1 = din("gW1", (COND, 3 * N))
    gb1 = din("gb1", (3 * N, 1))
    gW2 = din("gW2", (3 * N, N))
    gb2 = din("gb2", (1, N))
    out = nc.dram_tensor("out", (BH, OUT), F32, kind="ExternalOutput").ap()

    with tile.TileContext(nc) as tc:
        with tc.tile_pool(name="const", bufs=1) as cp, \
             tc.tile_pool(name="tmp", bufs=4) as tp:
            ident = cp.tile([128, 128], F32)
            make_identity(nc, ident)
            ones = cp.tile([1, 128], F32)
            nc.vector.memset(ones, 1.0)
            halfpi = cp.tile([128, 1], F32)
            nc.vector.memset(halfpi, math.pi / 2)

            sxh = cp.tile([128, 2, BH], F16)
            for c in range(2):
                nc.sync.dma_start(sxh[:, c, :], xh16[c])
            scT = cp.tile([COND, BH], F32)
            nc.sync.dma_start(scT, condT)
            swb = cp.tile([128, 2, W], F16)
            for c in range(2):
                nc.sync.dma_start(swb[:, c, :], wbase[c])
            sw2b = cp.tile([H + 1, 2, DN], F32)
            for e in range(2):
                nc.sync.dma_start(sw2b[:, e, :], w2b[e])
            shW1 = cp.tile([COND, H], F32)
            nc.sync.dma_start(shW1, hW1)
            shb1 = cp.tile([1, H], F32)
            nc.sync.dma_start(shb1, hb1)
            sgW1 = cp.tile([COND, 3 * N], F32)
            nc.sync.dma_start(sgW1, gW1)
            sgb1 = cp.tile([3 * N, 1], F32)
            nc.sync.dma_start(sgb1, gb1)
            sgW2 = cp.tile([3 * N, N], F32)
            nc.sync.dma_start(sgW2, gW2)
            sgb2 = cp.tile([1, N], F32)
            nc.sync.dma_start(sgb2, gb2)

            hTa = cp.tile([H + 1, NBT * 128], F32)
            nc.vector.memset(hTa[H:H + 1, :], 1.0)
            h_sb = [cp.tile([128, H], F32, name=f"h{t}") for t in range(NBT)]
            out_sb = [cp.tile([128, W], F32, name=f"os{t}") for t in range(NBT)]
            outf = [cp.tile([128, OUT], F32, name=f"of{t}") for t in range(NBT)]
            gw_sb = [cp.tile([128, 2], F32, name=f"gw{t}") for t in range(NBT)]
            acc = [cp.tile([128, SG - NSTT[t % 2], W], F16, name=f"ac{t}")
                   for t in range(NBT)]
            for t in range(NBT):
                nc.vector.memset(acc[t], 0.0)
            wt0 = None
            if "dma" in ablate:
                wt0 = cp.tile([128, SG * 768], F16, name="wt0")
                nc.vector.memset(wt0[:, 0:8], 0.25)

            # ---------------- prologue: gating, hypernet h, base ----------
            with tc.tile_pool(name="pps", bufs=2, space="PSUM") as pps:
                # dummy Exp first so walrus loads the exp table-set (which
                # also contains Relu/Copy) once; only Sin needs a 2nd load
                edum = tp.tile([1, 1], F32, tag="edum")
                nc.scalar.activation(edum, ones[:, 0:1], AF.Exp)

                g1 = pps.tile([3 * N, BH], F32, tag="g1", bufs=1)
                nc.tensor.matmul(g1, sgW1, scT, start=True, stop=True)
                g1s = cp.tile([3 * N, BH], F32)
                nc.scalar.activation(g1s, g1, AF.Relu, bias=sgb1)

                for bt in range(NBT):
                    bs = slice(bt * 128, bt * 128 + 128)
                    hp = pps.tile([128, H], F32, tag="hp", bufs=1)
                    nc.tensor.matmul(hp, scT[:, bs], shW1,
                                     start=True, stop=False)
                    nc.tensor.matmul(hp, ones, shb1, start=False, stop=True)
                    nc.scalar.activation(h_sb[bt], hp, AF.Relu)
                    ht = pps.tile([H, 128], F32, tag="ht", bufs=1)
                    nc.tensor.transpose(ht, h_sb[bt], ident)
                    nc.scalar.copy(hTa[0:H, bs], ht)

                    lg = pps.tile([128, N], F32, tag="lg", bufs=1)
                    nc.tensor.matmul(lg, g1s[:, bs], sgW2,
                                     start=True, stop=False)
                    nc.tensor.matmul(lg, ones, sgb2, start=False, stop=True)
                    nmx = tp.tile([128, 1], F32, tag="nmx")
                    nc.vector.tensor_reduce(nmx, lg, axis=mybir.AxisListType.X,
                                            op=OP.max, negate=True)
                    ex = tp.tile([128, N], F32, tag="ex")
                    nc.scalar.activation(ex, lg, AF.Exp, bias=nmx)
                    sm = tp.tile([128, 1], F32, tag="sm")
                    nc.vector.tensor_reduce(sm, ex, axis=mybir.AxisListType.X,
                                            op=OP.add)
                    rv = tp.tile([128, 1], F32, tag="rv")
                    nc.vector.reciprocal(rv, sm)
                    nc.vector.tensor_scalar_mul(gw_sb[bt], ex[:, 0:2], rv)

                    bp = pps.tile([128, W], F32, tag="bp", bufs=2)
                    nc.tensor.matmul(bp, sxh[:, 0, bs], swb[:, 0, :],
                                     start=True, stop=False)
                    nc.tensor.matmul(bp, sxh[:, 1, bs], swb[:, 1, :],
                                     start=False, stop=False)
                    nc.tensor.matmul(bp[:, 128:256], hTa[:, bs],
                                     sw2b[:, 0, :], start=False, stop=False)
                    nc.tensor.matmul(bp[:, 256:384], hTa[:, bs],
                                     sw2b[:, 1, :], start=False, stop=True)
                    nc.scalar.copy(out_sb[bt], bp)

                # dummy Sin that data-depends on the LAST gating exp: loads
                # the trig table set (which also holds Copy/Relu) exactly
                # once, after all Exp uses — no mid-loop or epilogue reloads
                sdum = tp.tile([128, 1], F32, tag="sdum")
                nc.scalar.activation(sdum, gw_sb[NBT - 1][:, 0:1], AF.Sin)



            # ---------------- main loop over hypernet unit groups ----------
            with tc.tile_pool(name="wp", bufs=3) as wp, \
                 tc.tile_pool(name="scp", bufs=4) as scp, \
                 tc.tile_pool(name="mps", bufs=8, space="PSUM") as mps:

              def _main_body():
                mm = nc.tensor.matmul
                for g in range(NSG):
                    k0 = g * SG
                    if "dma" not in ablate:
                        wt = wp.tile([128, SG * 768], F16, tag="w",
                                     name=f"w{g}")
                        nc.sync.dma_start(wt, w2g[g])
                    else:
                        wt = wt0
                    for bt in range(NBT):
                        bs = slice(bt * 128, bt * 128 + 128)
                        prs = [mps.tile([128, W], F32, tag="ps",
                                        name=f"ps{j}") for j in range(SG)]
                        if "mm" in ablate and "drain" not in ablate:
                            # cheap N=1 writes so drains have a producer
                            for j in range(SG):
                                mm(prs[j][:, 0:1], sxh[:, 0, bs],
                                   wt[:, j * 768:j * 768 + 1],
                                   start=True, stop=True)
                        if "mm" not in ablate:
                            # stationary-major: ic0 for all 8 k, then ic1.
                            # Issue the DVE-drained tiles (5,6,7) first:
                            # they are the earliest-freed banks of the
                            # previous phase, so the PE stalls least.
                            for j in MMORD:
                                mm(prs[j], sxh[:, 0, bs],
                                   wt[:, j * 768:j * 768 + 384],
                                   start=True, stop=False)
                            for j in MMORD:
                                mm(prs[j], sxh[:, 1, bs],
                                   wt[:, j * 768 + 384:j * 768 + 768],
                                   start=False, stop=True)
                        if "drain" in ablate:
                            continue
                        ns = NSTT[bt % 2]
                        na = SG - ns
                        # stt tiles are the LAST ns of {5,6,7}; ACT tiles
                        # are the rest, drained in an order matching the
                        # next phase's MM issue order (tile 5 first when
                        # it is ACT-role, so its bank frees early)
                        stt_tiles = list(range(SG - ns, SG))
                        act_tiles = [t for t in MMORD if t not in stt_tiles]
                        sg = scp.tile([128, na, W], F16, tag=f"sc{bt % 2}",
                                      name=f"sc{bt % 2}")
                        for s, t in enumerate(act_tiles):
                            nc.scalar.activation(
                                sg[:, s, :], prs[t], AF.Copy,
                                scale=h_sb[bt][:, k0 + t:k0 + t + 1])
                        # direct DVE stt drains (emitted before the grouped
                        # add: DVE is FIFO and the add must wait for the
                        # whole ACT chain)
                        for t in stt_tiles:
                            nc.vector.scalar_tensor_tensor(
                                out_sb[bt], prs[t],
                                h_sb[bt][:, k0 + t:k0 + t + 1],
                                out_sb[bt], op0=OP.mult, op1=OP.add)
                        # grouped add on the otherwise-idle GPSIMD engine
                        # (~5us/op vs 6.6us phase spacing for this bt, so it
                        # keeps pace and frees ~25us of DVE time)
                        nc.gpsimd.tensor_add(acc[bt], acc[bt], sg)

              if repeat_main == 1:
                  _main_body()
              else:
                  with tc.For_i(0, repeat_main, 1):
                      _main_body()

              # merge the fp16 accumulator slots into out_sb (tree reduce)
              for bt in range(NBT):
                  slots = [acc[bt][:, j, :] for j in range(SG - NSTT[bt % 2])]
                  lvl = 0
                  while len(slots) > 1:
                      nxt = []
                      for i in range(0, len(slots) - 1, 2):
                          t1 = tp.tile([128, W], F16, tag=f"tr{bt}{lvl}{i}")
                          nc.vector.tensor_add(t1, slots[i], slots[i + 1])
                          nxt.append(t1)
                      if len(slots) % 2:
                          nxt.append(slots[-1])
                      slots = nxt
                      lvl += 1
                  nc.vector.tensor_add(out_sb[bt], out_sb[bt], slots[0])

            # ---------------- epilogue: sin/cos/relu, gate, store ----------
            def sin_reduced(v, outname, fd, quarter, bias, out_ap=None):
                """sin(v + bias) via range reduction: r = v - 2pi*rint(
                v/2pi + quarter); the bias is applied for free inside the
                ACT Sin evaluation (spline arg = r + bias in [-pi, pi])."""
                t1 = tp.tile([128, fd], F32, tag="t1")
                if quarter:
                    nc.vector.tensor_scalar(t1, v, INV2PI, quarter,
                                            op0=OP.mult, op1=OP.add)
                else:
                    nc.vector.tensor_scalar_mul(t1, v, INV2PI)
                ti = tp.tile([128, fd], dt.int32, tag="ti")
                nc.vector.tensor_copy(ti, t1)
                tf = tp.tile([128, fd], F32, tag="tf")
                nc.vector.tensor_copy(tf, ti)
                r = tp.tile([128, fd], F32, tag="r")
                nc.vector.scalar_tensor_tensor(r, tf, N2PI, v,
                                               op0=OP.mult, op1=OP.add)
                if GUARDS:
                    m = tp.tile([128, fd], F32, tag="m")
                    nc.vector.tensor_scalar(m, r, math.pi - bias, None,
                                            op0=OP.is_gt)
                    nc.vector.scalar_tensor_tensor(r, m, N2PI, r,
                                                   op0=OP.mult, op1=OP.add)
                    nc.vector.tensor_scalar(m, r, -math.pi - bias, None,
                                            op0=OP.is_lt)
                    nc.vector.scalar_tensor_tensor(r, m, -N2PI, r,
                                                   op0=OP.mult, op1=OP.add)
                if out_ap is None:
                    out_ap = tp.tile([128, fd], F32, tag=outname, name=outname)
                nc.scalar.activation(out_ap, r, AF.Sin,
                                     bias=halfpi if bias else 0.0)
                return out_ap

            for bt in range(NBT):
                # staging tile: per expert e, [cv (64) | sv (64) | nn (128)]
                # matching the output column layout, so gating is 2 ops
                et = tp.tile([128, 2, OUT], F32, tag="et", name=f"et{bt}")
                # both experts' periodic pre-activations at once (128 cols)
                th2 = out_sb[bt][:, 0:2 * DP]
                sin_reduced(th2, "sv", 2 * DP, 0.0, 0.0,
                            out_ap=et[:, :, DP:2 * DP])
                # cos(v) = sin(v + pi/2); fold pi/2 into the int-round
                # (quarter=0.25) and the ACT bias
                sin_reduced(th2, "cv", 2 * DP, 0.25, math.pi / 2,
                            out_ap=et[:, :, 0:DP])
                nc.scalar.activation(et[:, :, 2 * DP:OUT],
                                     out_sb[bt][:, 2 * DP:W], AF.Relu)

                nc.vector.tensor_scalar_mul(outf[bt], et[:, 0, :],
                                            gw_sb[bt][:, 0:1])
                nc.vector.scalar_tensor_tensor(
                    outf[bt], et[:, 1, :], gw_sb[bt][:, 1:2], outf[bt],
                    op0=OP.mult, op1=OP.add)
                nc.sync.dma_start(out[bt * 128:bt * 128 + 128, :], outf[bt])

    nc.finalize()
    return nc


def _host_prep(x, cond, base_wp, base_wn, base_bn, hW1, hb1, hW2, hb2,
               gW1, gb1, gW2, gb2):
    """Build the 8 per-core input maps (layout prep + sharding only)."""
    f32 = np.float32
    W2r = np.asarray(hW2, f32).reshape(H, N, TPE)
    wpW = W2r[:, :, :IN * DP].reshape(H, N, IN, DP)
    wnW = W2r[:, :, IN * DP:IN * DP + IN * DN].reshape(H, N, IN, DN)
    bnW = W2r[:, :, IN * DP + IN * DN:]                    # (H, N, DN)
    hb2r = np.asarray(hb2, f32).reshape(N, TPE)
    hwp = hb2r[:, :IN * DP].reshape(N, IN, DP)
    hwn = hb2r[:, IN * DP:IN * DP + IN * DN].reshape(N, IN, DN)
    hbn = hb2r[:, IN * DP + IN * DN:]                      # (N, DN)

    base_wp = np.asarray(base_wp, f32)
    base_wn = np.asarray(base_wn, f32)
    base_bn = np.asarray(base_bn, f32)
    x = np.asarray(x, f32)
    cond = np.asarray(cond, f32)
    gW2 = np.asarray(gW2, f32)
    gb2 = np.asarray(gb2, f32)

    common = dict(
        hW1=np.ascontiguousarray(hW1, f32),
        hb1=np.asarray(hb1, f32).reshape(1, H).copy(),
        gW1=np.ascontiguousarray(gW1, f32),
        gb1=np.asarray(gb1, f32).reshape(3 * N, 1).copy(),
    )

    # per batch-half arrays
    halves = []
    for hb in range(2):
        bs = slice(hb * BH, (hb + 1) * BH)
        xT = np.ascontiguousarray(x[bs].T)                 # (IN, BH)
        xh = xT.astype(np.float16)
        halves.append(dict(
            xh16=np.ascontiguousarray(xh.reshape(2, 128, BH)),
            condT=np.ascontiguousarray(cond[bs].T),
        ))

    pairs = []
    for p in range(4):
        e0, e1 = 2 * p, 2 * p + 1
        wpcat = np.concatenate([wpW[:, e0], wpW[:, e1]], axis=-1)  # (H,IN,128)
        ncat = np.concatenate([wnW[:, e0], wnW[:, e1]], axis=-1)   # (H,IN,256)
        wpc = wpcat.reshape(H, 2, 128, 128).astype(np.float16)
        nc_ = ncat.reshape(H, 2, 128, 256).astype(np.float16)
        # per k: [wp ic0 | wn ic0 | wp ic1 | wn ic1]  (768 cols fp16)
        w2h = np.concatenate(
            [wpc[:, 0], nc_[:, 0], wpc[:, 1], nc_[:, 1]], axis=-1)
        # group by SG: (NSG, 128, SG*768) so each group DMA is contiguous
        w2g = np.ascontiguousarray(
            w2h.reshape(NSG, SG, 128, 768).transpose(0, 2, 1, 3)
               .reshape(NSG, 128, SG * 768))
        w2b = np.stack([
            np.concatenate([bnW[:, e], (base_bn[e] + hbn[e])[None, :]], axis=0)
            for e in (e0, e1)])                            # (2, 65, DN)
        wb = np.concatenate(
            [base_wp[e0] + hwp[e0], base_wp[e1] + hwp[e1],
             base_wn[e0] + hwn[e0], base_wn[e1] + hwn[e1]],
            axis=-1)                                       # (IN, 384)
        perm = [e0, e1] + [j for j in range(N) if j not in (e0, e1)]
        pairs.append(dict(
            w2g=w2g,
            w2b=np.ascontiguousarray(w2b),
            wbase=np.ascontiguousarray(
                wb.reshape(2, 128, W).astype(np.float16)),
            gW2=np.ascontiguousarray(gW2[:, perm]),
            gb2=np.ascontiguousarray(gb2[perm].reshape(1, N)),
        ))

    in_maps = []
    for c in range(8):
        p, hb = c // 2, c % 2
        m = dict(common)
        m.update(halves[hb])
        m.update(pairs[p])
        in_maps.append(m)
    return in_maps


def _make_runner(nc, n_cores=8):
    """Compile once; reusable executor for per-core input maps."""
    import jax
    from jax.sharding import Mesh, PartitionSpec
    from jax.experimental.shard_map import shard_map
    from concourse.bass2jax import (_bass_exec_p, install_neuronx_cc_hook,
                                    partition_id_tensor)

    install_neuronx_cc_hook()
    pname = nc.partition_id_tensor.name if nc.partition_id_tensor else None
    in_names, out_names, out_avals, zero_outs = [], [], [], []
    for alloc in nc.m.functions[0].allocations:
        if not isinstance(alloc, mybir.MemoryLocationSet):
            continue
        name = alloc.memorylocations[0].name
        if alloc.kind == "ExternalInput":
            if name != pname:
                in_names.append(name)
        elif alloc.kind == "ExternalOutput":
            out_names.append(name)
            shape = tuple(alloc.tensor_shape)
            dtype = mybir.dt.np(alloc.dtype)
            out_avals.append(jax.core.ShapedArray(shape, dtype))
            zero_outs.append(np.zeros(shape, dtype))
    n_params = len(in_names)
    n_outs = len(out_avals)
    all_names = in_names + out_names + ([pname] if pname else [])

    def _body(*args):
        operands = list(args)
        if pname is not None:
            operands.append(partition_id_tensor())
        outs = _bass_exec_p.bind(
            *operands, out_avals=tuple(out_avals), in_names=tuple(all_names),
            out_names=tuple(out_names), lowering_input_output_aliases=(),
            sim_require_finite=True, sim_require_nnan=True, nc=nc)
        return tuple(outs)

    devices = jax.devices()[:n_cores]
    mesh = Mesh(np.asarray(devices), ("core",))
    in_specs = (PartitionSpec("core"),) * (n_params + n_outs)
    out_specs = (PartitionSpec("core"),) * n_outs
    donate = tuple(range(n_params, n_params + n_outs))
    sharded = jax.jit(
        shard_map(_body, mesh=mesh, in_specs=in_specs, out_specs=out_specs,
                  check_rep=False),
        donate_argnums=donate, keep_unused=True)

    staged = {}

    def _concat(in_maps):
        return [
            np.concatenate([np.asarray(in_maps[c][in_names[i]])
                            for c in range(n_cores)], axis=0)
            for i in range(n_params)
        ]

    def run(in_maps):
        if in_maps is None:
            concat_in = staged["dev"]
        else:
            concat_in = _concat(in_maps)
        zeros = [np.zeros((n_cores * z.shape[0], *z.shape[1:]), z.dtype)
                 for z in zero_outs]
        outs = sharded(*concat_in, *zeros)
        arr = np.asarray(outs[0]).reshape(n_cores, *out_avals[0].shape)
        return [{out_names[0]: arr[c]} for c in range(n_cores)]

    def preload(in_maps):
        import jax
        staged["dev"] = [jax.device_put(a) for a in _concat(in_maps)]
        for a in staged["dev"]:
            a.block_until_ready()

    run.preload = preload
    return run


def kernel(**inputs):
    if "run" not in _cache:
        nc = _build()
        _cache["nc"] = nc
        _cache["run"] = _make_runner(nc)
    in_maps = _host_prep(**inputs)
    results = _cache["run"](in_maps)
    out = np.zeros((B, OUT), np.float32)
    for c in range(8):
        hb = c % 2
        out[hb * BH:(hb + 1) * BH] += results[c]["out"]
    return out
